# revision 3
# baseline (speedup 1.0000x reference)
"""Trainium2 Bass kernel v3 for nn_CrossAttention (B=2, T=2048, D=1024, H=16, hd=64).

Sharding: core c handles batch c//4, heads (c%4)*4 .. +4 (4 heads, 256 dims).

v3: attention in 8 passes (head-pair j x 512-wide q-chunk). S tiles hold two
kv-blocks [128, 2, 512] so exp stays [128,1024]; y accumulators [65, 512]
(1 bank). PSUM: psS 2x2 banks, psY 3x1, psF 1x1. q-projection and c_proj
chunks are injected through psF into the attention stream (PE stays fed).

dtypes: S operands bf16; es/vext f32r; PSUM f32; out bf16.
softmax: ones-column in vext -> denominator row 64; SBUF->SBUF DMA moves it to
partition 0; reciprocal_approx_fast (DVE); partition_broadcast (GPSIMD);
normalize mult (DVE) writes yallT bf16.  Host sums 4 core partials + bc.
"""

import sys

sys.path.insert(0, "/opt/trn_rl_repo")

import numpy as np
import ml_dtypes

import concourse.bacc as bacc
import concourse.bass as bass
import concourse.mybir as mybir
import concourse.tile as tile
from concourse.bass_utils import run_bass_kernel_spmd

F32 = mybir.dt.float32
F32R = mybir.dt.float32r
BF16 = mybir.dt.bfloat16
EXP = mybir.ActivationFunctionType.Exp
ADD = mybir.AluOpType.add
MULT = mybir.AluOpType.mult

T = 2048
D = 1024
HL = 4
HD = 64
DH = HL * HD      # 256
P = 128
KT = D // P       # 8
JT = DH // P      # 2
QC = 512
NQC = T // QC     # 4
NMV = T // (2 * P)  # 8
SCALE = 0.125
N_CORES = 8

_cache = {}


def build_nc():
    if "nc" in _cache:
        return _cache["nc"]
    nc = bacc.Bacc(
        "TRN2",
        target_bir_lowering=False,
        debug=False,
        num_devices=N_CORES,
    )

    qT = nc.declare_dram_parameter("qT", [D, T], BF16, isOutput=False)
    kT = nc.declare_dram_parameter("kT", [D, T], BF16, isOutput=False)
    v_sl = nc.declare_dram_parameter("v_sl", [T, DH], F32R, isOutput=False)
    WqT = nc.declare_dram_parameter("WqT", [D, DH], BF16, isOutput=False)
    WkT = nc.declare_dram_parameter("WkT", [D, DH], BF16, isOutput=False)
    WcT = nc.declare_dram_parameter("WcT", [DH, D], BF16, isOutput=False)
    bqk = nc.declare_dram_parameter("bqk", [P, 4], F32, isOutput=False)
    onescol = nc.declare_dram_parameter("onescol", [P, T // P], F32R, isOutput=False)
    out = nc.declare_dram_parameter("out", [T, D], BF16, isOutput=True)

    with tile.TileContext(nc) as tc:
        with (
            tc.tile_pool(name="wpool", bufs=1) as wpool,
            tc.tile_pool(name="xpool", bufs=1) as xpool,
            tc.tile_pool(name="projsb", bufs=1) as projsb,
            tc.tile_pool(name="vpool", bufs=1) as vpool,
            tc.tile_pool(name="epool", bufs=4) as epool,
            tc.tile_pool(name="npool", bufs=2) as npool,
            tc.tile_pool(name="bpool", bufs=3) as bpool,
            tc.tile_pool(name="opool", bufs=4) as opool,
            tc.tile_pool(name="psS", bufs=2, space="PSUM") as psS,
            tc.tile_pool(name="psY", bufs=3, space="PSUM") as psY,
            tc.tile_pool(name="psF", bufs=1, space="PSUM") as psF,
        ):
            # ---- staging: order DMAs by first use ----
            wk_sb = wpool.tile([P, KT, DH], BF16, name="wk_sb")
            nc.sync.dma_start(wk_sb[:], WkT.ap().rearrange("(a p) m -> p a m", p=P))
            bias_sb = wpool.tile([P, 4], F32, name="bias_sb")  # [bq0,bq1,bk0,bk1]
            nc.sync.dma_start(bias_sb[:], bqk.ap())
            kt_sb = xpool.tile([P, KT, T], BF16, name="kt_sb")
            wq_sb = wpool.tile([P, KT, DH], BF16, name="wq_sb")
            qt_sb = xpool.tile([P, KT, T], BF16, name="qt_sb")
            for i in range(KT):
                nc.sync.dma_start(kt_sb[:, i, :], kT.ap()[i * P:(i + 1) * P, :])
            nc.sync.dma_start(wq_sb[:], WqT.ap().rearrange("(a p) m -> p a m", p=P))
            # qT: qc0 columns first (needed by first attention pass)
            for i in range(KT):
                nc.sync.dma_start(qt_sb[:, i, 0:QC], qT.ap()[i * P:(i + 1) * P, 0:QC])
            v_re = v_sl.ap().rearrange("(t p) d -> p t d", p=P)  # [128, 16, 256]
            vext = []
            for h in range(HL):
                ve = vpool.tile([P, T // P, HD + 1], F32R, name=f"vext{h}")
                nc.sync.dma_start(ve[:, :, 0:HD], v_re[:, :, h * HD:(h + 1) * HD])
                nc.sync.dma_start(ve[:, :, HD:HD + 1], onescol.ap().unsqueeze(2))
                vext.append(ve)
            for i in range(KT):
                nc.sync.dma_start(qt_sb[:, i, QC:T], qT.ap()[i * P:(i + 1) * P, QC:T])
            wc_sb = wpool.tile([P, JT, D], BF16, name="wc_sb")
            nc.sync.dma_start(wc_sb[:], WcT.ap().rearrange("(a p) m -> p a m", p=P))

            # HAM warmup: dep-free matmuls on wk_sb while inputs stream in
            for wi in range(8):
                wps = psF.tile([P, QC], F32, tag="f", name=f"warm{wi}")
                nc.tensor.matmul(wps[:, 0:DH], wk_sb[:, 0, 0:P],
                                 wk_sb[:, wi % KT, 0:DH],
                                 start=True, stop=True)

            kpT = projsb.tile([P, JT, T], BF16, name="kpT")
            qpT = projsb.tile([P, JT, T], BF16, name="qpT")
            yallT = projsb.tile([P, JT, T], BF16, name="yallT")

            # ---- upfront projections through psS pair tiles ----
            def proj_pair(jobs, name):
                # jobs: list of (xt_sb, w_sb, bias_col0, xpT, j, tc5), len<=2
                psP = psS.tile([P, 2, QC], F32, tag="ps", name=f"pp{name}")
                for idx, (xt_sb, w_sb, b0, xpT, j, tc5) in enumerate(jobs):
                    for i in range(KT):
                        nc.tensor.matmul(
                            psP[:, idx, :],
                            w_sb[:, i, j * P:(j + 1) * P],
                            xt_sb[:, i, tc5 * QC:(tc5 + 1) * QC],
                            start=(i == 0),
                            stop=(i == KT - 1),
                        )
                for idx, (xt_sb, w_sb, b0, xpT, j, tc5) in enumerate(jobs):
                    nc.vector.tensor_tensor(
                        xpT[:, j, tc5 * QC:(tc5 + 1) * QC],
                        psP[:, idx, :],
                        bias_sb[:, b0 + j:b0 + j + 1].to_broadcast((P, QC)),
                        ADD,
                    )

            kj = lambda j, t: (kt_sb, wk_sb, 2, kpT, j, t)
            qj = lambda j, t: (qt_sb, wq_sb, 0, qpT, j, t)
            proj_pair([kj(0, 0), kj(0, 1)], "k00")
            proj_pair([kj(0, 2), kj(0, 3)], "k01")
            proj_pair([qj(0, 0), qj(1, 0)], "q0")

            # ---- filler chunk emitters (one [128,512] psF tile each) ----
            def k_chunk(j, tc5):
                def emit():
                    psP = psF.tile([P, QC], F32, tag="f", name=f"kf{j}_{tc5}")
                    for i in range(KT):
                        nc.tensor.matmul(
                            psP[:],
                            wk_sb[:, i, j * P:(j + 1) * P],
                            kt_sb[:, i, tc5 * QC:(tc5 + 1) * QC],
                            start=(i == 0),
                            stop=(i == KT - 1),
                        )
                    nc.vector.tensor_tensor(
                        kpT[:, j, tc5 * QC:(tc5 + 1) * QC],
                        psP[:],
                        bias_sb[:, 2 + j:2 + j + 1].to_broadcast((P, QC)),
                        ADD,
                    )
                return emit

            def q_chunk(j, tc5):
                def emit():
                    psP = psF.tile([P, QC], F32, tag="f", name=f"qf{j}_{tc5}")
                    for i in range(KT):
                        nc.tensor.matmul(
                            psP[:],
                            wq_sb[:, i, j * P:(j + 1) * P],
                            qt_sb[:, i, tc5 * QC:(tc5 + 1) * QC],
                            start=(i == 0),
                            stop=(i == KT - 1),
                        )
                    nc.vector.tensor_tensor(
                        qpT[:, j, tc5 * QC:(tc5 + 1) * QC],
                        psP[:],
                        bias_sb[:, j:j + 1].to_broadcast((P, QC)),
                        ADD,
                    )
                return emit

            def cproj_chunk(mt, cc):
                def emit():
                    o_ps = psF.tile([P, QC], F32, tag="f", name=f"o{mt}_{cc}")
                    for j in range(JT):
                        nc.tensor.matmul(
                            o_ps[:],
                            yallT[:, j, mt * P:(mt + 1) * P],
                            wc_sb[:, j, cc * QC:(cc + 1) * QC],
                            start=(j == 0),
                            stop=(j == JT - 1),
                        )
                    o_sb = opool.tile([P, QC], BF16, tag="osb",
                                      name=f"ot{mt}_{cc}")
                    nc.vector.tensor_copy(o_sb[:], o_ps[:])
                    nc.sync.dma_start(
                        out.ap()[mt * P:(mt + 1) * P, cc * QC:(cc + 1) * QC],
                        o_sb[:],
                    )
                return emit

            def cproj_pair(mt):
                # post-attention: both halves of a row-block via one psS tile
                psP = psS.tile([P, 2, QC], F32, tag="ps", name=f"cp{mt}")
                for cc in range(2):
                    for j in range(JT):
                        nc.tensor.matmul(
                            psP[:, cc, :],
                            yallT[:, j, mt * P:(mt + 1) * P],
                            wc_sb[:, j, cc * QC:(cc + 1) * QC],
                            start=(j == 0),
                            stop=(j == JT - 1),
                        )
                o_sb = opool.tile([P, 2, QC], BF16, tag="osb2", name=f"otp{mt}")
                nc.scalar.copy(o_sb[:], psP[:])
                nc.sync.dma_start(out.ap()[mt * P:(mt + 1) * P, :], o_sb[:])

            # ---- attention pass with fillers ----
            def attn_pass(j, qc, fillers):
                q0 = qc * QC

                def s_mms(mv):
                    pair = []
                    for ab in range(2):
                        s = psS.tile([P, 2, QC], F32, tag="ps",
                                     name=f"s{j}{qc}{mv}{ab}")
                        p0 = ab * HD
                        for sub in range(2):
                            kv = mv * 2 + sub
                            nc.tensor.matmul(
                                s[:, sub, :],
                                kpT[p0:p0 + HD, j, kv * P:(kv + 1) * P],
                                qpT[p0:p0 + HD, j, q0:q0 + QC],
                                start=True,
                                stop=True,
                                tile_position=(p0, 0),
                            )
                        pair.append(s)
                    return pair

                yps = [psY.tile([HD + 1, QC], F32, tag="y",
                                name=f"y{j}{qc}{ab}") for ab in range(2)]
                fl = list(fillers)
                fidx = 0
                pend = s_mms(0)
                for mv in range(NMV):
                    nxt = s_mms(mv + 1) if mv + 1 < NMV else None
                    for ab in range(2):
                        es = epool.tile([P, 2, QC], F32R, tag="es",
                                        name=f"e{j}{qc}{mv}{ab}")
                        nc.scalar.activation(es[:], pend[ab][:], EXP, scale=SCALE)
                        for sub in range(2):
                            kv = mv * 2 + sub
                            nc.tensor.matmul(
                                yps[ab][:],
                                vext[2 * j + ab][:, kv, :],
                                es[:, sub, :],
                                start=(mv == 0 and sub == 0),
                                stop=(mv == NMV - 1 and sub == 1),
                            )
                    pend = nxt
                    if fidx < len(fl):
                        fl[fidx]()
                        fidx += 1
                while fidx < len(fl):
                    fl[fidx]()
                    fidx += 1
                # normalize both heads of the pair
                for ab in range(2):
                    p0 = ab * HD
                    y_ps = yps[ab]
                    dsc = npool.tile([HD + 1, QC], F32, tag="dsc",
                                     name=f"d{j}{qc}{ab}")
                    nc.vector.tensor_copy(dsc[HD:HD + 1, :], y_ps[HD:HD + 1, :])
                    dn0 = bpool.tile([1, QC], F32, tag="dn0",
                                     name=f"dn{j}{qc}{ab}")
                    nc.sync.dma_start(dn0[:], dsc[HD:HD + 1, :])
                    rc = bpool.tile([1, QC], F32, tag="rc",
                                    name=f"rc{j}{qc}{ab}")
                    nc.vector.reciprocal_approx_fast(rc[:], dn0[:])
                    bc = bpool.tile([HD, QC], F32, tag="bc",
                                    name=f"b{j}{qc}{ab}")
                    nc.gpsimd.partition_broadcast(bc[:], rc[:])
                    nc.vector.tensor_tensor(
                        yallT[p0:p0 + HD, j, q0:q0 + QC],
                        y_ps[0:HD, :], bc[:], MULT,
                    )

            # ---- filler schedule over the 8 passes ----
            filler_lists = [
                [k_chunk(1, 0), k_chunk(1, 1), k_chunk(1, 2), k_chunk(1, 3),
                 q_chunk(0, 1)],
                [q_chunk(1, 1), q_chunk(0, 2), q_chunk(1, 2)],
                [cproj_chunk(0, 0), cproj_chunk(0, 1),
                 cproj_chunk(1, 0), cproj_chunk(1, 1)],
                [cproj_chunk(2, 0), cproj_chunk(2, 1),
                 cproj_chunk(3, 0), cproj_chunk(3, 1)],
                [q_chunk(0, 3), q_chunk(1, 3),
                 cproj_chunk(4, 0), cproj_chunk(4, 1)],
                [cproj_chunk(5, 0), cproj_chunk(5, 1),
                 cproj_chunk(6, 0), cproj_chunk(6, 1)],
                [cproj_chunk(7, 0), cproj_chunk(7, 1),
                 cproj_chunk(8, 0), cproj_chunk(8, 1)],
                [cproj_chunk(9, 0), cproj_chunk(9, 1),
                 cproj_chunk(10, 0), cproj_chunk(10, 1)],
            ]

            pi = 0
            for qc in range(NQC):
                for j in range(JT):
                    attn_pass(j, qc, filler_lists[pi])
                    pi += 1

            # tail: remaining cproj row-blocks
            for mt in (11, 12, 13, 14, 15):
                cproj_pair(mt)

    nc.compile()
    _cache["nc"] = nc
    return nc


def _bf(x):
    return np.ascontiguousarray(np.asarray(x, dtype=ml_dtypes.bfloat16))


def make_in_maps(k, q, v, Wk, bk, Wq, bq, Wc, bc):
    k = np.asarray(k, dtype=np.float32)
    q = np.asarray(q, dtype=np.float32)
    v = np.asarray(v, dtype=np.float32)
    Wk = np.asarray(Wk, dtype=np.float32)
    Wq = np.asarray(Wq, dtype=np.float32)
    Wc = np.asarray(Wc, dtype=np.float32)
    bk = np.asarray(bk, dtype=np.float32)
    bq = np.asarray(bq, dtype=np.float32)
    in_maps = []
    for c in range(N_CORES):
        b = c // 4
        h0 = (c % 4) * HL
        sl = slice(h0 * HD, h0 * HD + DH)
        bq_t = np.ascontiguousarray(bq[sl].reshape(2, P).T)  # [128, 2]
        bk_t = np.ascontiguousarray(bk[sl].reshape(2, P).T)
        bqk_m = np.concatenate([bq_t, bk_t], axis=1)         # [128, 4]
        in_maps.append({
            "qT": _bf(q[b].T),
            "kT": _bf(k[b].T),
            "v_sl": np.ascontiguousarray(v[b][:, sl]),
            "WqT": _bf(Wq[sl, :].T),
            "WkT": _bf(Wk[sl, :].T),
            "WcT": _bf(Wc[:, sl].T),
            "bqk": np.ascontiguousarray(bqk_m),
            "onescol": np.ones((P, T // P), dtype=np.float32),
        })
    return in_maps


def kernel(k, q, v, Wk, bk, Wq, bq, Wc, bc, _trace=False, _trace_cores=None):
    bc = np.asarray(bc, dtype=np.float32)
    nc = build_nc()
    in_maps = make_in_maps(k, q, v, Wk, bk, Wq, bq, Wc, bc)
    res = run_bass_kernel_spmd(
        nc, in_maps, core_ids=list(range(N_CORES)),
        trace=_trace, trace_cores=_trace_cores,
    )
    outs = [res.results[c]["out"].astype(np.float32) for c in range(N_CORES)]
    full = np.stack([
        outs[0] + outs[1] + outs[2] + outs[3],
        outs[4] + outs[5] + outs[6] + outs[7],
    ]) + bc[None, None, :]
    kernel.last_result = res
    return full.astype(np.float32)


# revision 4
# speedup vs baseline: 1.0270x; 1.0270x over previous
"""Trainium2 Bass kernel v3 for nn_CrossAttention (B=2, T=2048, D=1024, H=16, hd=64).

Sharding: core c handles batch c//4, heads (c%4)*4 .. +4 (4 heads, 256 dims).

v3: attention in 8 passes (head-pair j x 512-wide q-chunk). S tiles hold two
kv-blocks [128, 2, 512] so exp stays [128,1024]; y accumulators [65, 512]
(1 bank). PSUM: psS 2x2 banks, psY 3x1, psF 1x1. q-projection and c_proj
chunks are injected through psF into the attention stream (PE stays fed).

dtypes: S operands bf16; es/vext f32r; PSUM f32; out bf16.
softmax: ones-column in vext -> denominator row 64; SBUF->SBUF DMA moves it to
partition 0; reciprocal_approx_fast (DVE); partition_broadcast (GPSIMD);
normalize mult (DVE) writes yallT bf16.  Host sums 4 core partials + bc.
"""

import sys

sys.path.insert(0, "/opt/trn_rl_repo")

import numpy as np
import ml_dtypes

import concourse.bacc as bacc
import concourse.bass as bass
import concourse.mybir as mybir
import concourse.tile as tile
from concourse.bass_utils import run_bass_kernel_spmd

F32 = mybir.dt.float32
F32R = mybir.dt.float32r
BF16 = mybir.dt.bfloat16
EXP = mybir.ActivationFunctionType.Exp
ADD = mybir.AluOpType.add
MULT = mybir.AluOpType.mult

T = 2048
D = 1024
HL = 4
HD = 64
DH = HL * HD      # 256
P = 128
KT = D // P       # 8
JT = DH // P      # 2
QC = 512
NQC = T // QC     # 4
NMV = T // (2 * P)  # 8
SCALE = 0.125
N_CORES = 8

_cache = {}


def build_nc():
    if "nc" in _cache:
        return _cache["nc"]
    nc = bacc.Bacc(
        "TRN2",
        target_bir_lowering=False,
        debug=False,
        num_devices=N_CORES,
    )

    qT = nc.declare_dram_parameter("qT", [D, T], BF16, isOutput=False)
    kT = nc.declare_dram_parameter("kT", [D, T], BF16, isOutput=False)
    v_sl = nc.declare_dram_parameter("v_sl", [T, DH], F32R, isOutput=False)
    WqT = nc.declare_dram_parameter("WqT", [D, DH], BF16, isOutput=False)
    WkT = nc.declare_dram_parameter("WkT", [D, DH], BF16, isOutput=False)
    WcT = nc.declare_dram_parameter("WcT", [DH, D], BF16, isOutput=False)
    bqk = nc.declare_dram_parameter("bqk", [P, 4], F32, isOutput=False)
    onescol = nc.declare_dram_parameter("onescol", [P, T // P], F32R, isOutput=False)
    out = nc.declare_dram_parameter("out", [T, D], BF16, isOutput=True)

    with tile.TileContext(nc) as tc:
        with (
            tc.tile_pool(name="wpool", bufs=1) as wpool,
            tc.tile_pool(name="xpool", bufs=1) as xpool,
            tc.tile_pool(name="projsb", bufs=1) as projsb,
            tc.tile_pool(name="vpool", bufs=1) as vpool,
            tc.tile_pool(name="epool", bufs=4) as epool,
            tc.tile_pool(name="npool", bufs=2) as npool,
            tc.tile_pool(name="bpool", bufs=3) as bpool,
            tc.tile_pool(name="opool", bufs=4) as opool,
            tc.tile_pool(name="psS", bufs=2, space="PSUM") as psS,
            tc.tile_pool(name="psY", bufs=3, space="PSUM") as psY,
            tc.tile_pool(name="psF", bufs=1, space="PSUM") as psF,
        ):
            # ---- staging: order DMAs by first use ----
            wk_sb = wpool.tile([P, KT, DH], BF16, name="wk_sb")
            nc.sync.dma_start(wk_sb[:], WkT.ap().rearrange("(a p) m -> p a m", p=P))
            bias_sb = wpool.tile([P, 4], F32, name="bias_sb")  # [bq0,bq1,bk0,bk1]
            nc.sync.dma_start(bias_sb[:], bqk.ap())
            kt_sb = xpool.tile([P, KT, T], BF16, name="kt_sb")
            wq_sb = wpool.tile([P, KT, DH], BF16, name="wq_sb")
            qt_sb = xpool.tile([P, KT, T], BF16, name="qt_sb")
            # kT in column halves (2KB lines): k-proj pair (t0,t1) starts ~6us earlier
            for i in range(KT):
                nc.sync.dma_start(kt_sb[:, i, 0:T // 2],
                                  kT.ap()[i * P:(i + 1) * P, 0:T // 2])
            nc.sync.dma_start(wq_sb[:], WqT.ap().rearrange("(a p) m -> p a m", p=P))
            for i in range(KT):
                nc.sync.dma_start(kt_sb[:, i, T // 2:T],
                                  kT.ap()[i * P:(i + 1) * P, T // 2:T])
            # qT: qc0 columns first (needed by first attention pass)
            for i in range(KT):
                nc.sync.dma_start(qt_sb[:, i, 0:QC], qT.ap()[i * P:(i + 1) * P, 0:QC])
            v_re = v_sl.ap().rearrange("(t p) d -> p t d", p=P)  # [128, 16, 256]
            vext = []
            for h in range(HL):
                ve = vpool.tile([P, T // P, HD + 1], F32R, name=f"vext{h}")
                nc.sync.dma_start(ve[:, :, 0:HD], v_re[:, :, h * HD:(h + 1) * HD])
                nc.sync.dma_start(ve[:, :, HD:HD + 1], onescol.ap().unsqueeze(2))
                vext.append(ve)
            for i in range(KT):
                nc.sync.dma_start(qt_sb[:, i, QC:T], qT.ap()[i * P:(i + 1) * P, QC:T])
            wc_sb = wpool.tile([P, JT, D], BF16, name="wc_sb")
            nc.sync.dma_start(wc_sb[:], WcT.ap().rearrange("(a p) m -> p a m", p=P))

            # HAM warmup: dep-free matmuls on wk_sb while inputs stream in
            for wi in range(8):
                wps = psF.tile([P, QC], F32, tag="f", name=f"warm{wi}")
                nc.tensor.matmul(wps[:, 0:DH], wk_sb[:, 0, 0:P],
                                 wk_sb[:, wi % KT, 0:DH],
                                 start=True, stop=True)

            kpT = projsb.tile([P, JT, T], BF16, name="kpT")
            qpT = projsb.tile([P, JT, T], BF16, name="qpT")
            yallT = projsb.tile([P, JT, T], BF16, name="yallT")

            # ---- upfront projections through psS pair tiles ----
            def proj_pair(jobs, name):
                # jobs: list of (xt_sb, w_sb, bias_col0, xpT, j, tc5), len<=2
                psP = psS.tile([P, 2, QC], F32, tag="ps", name=f"pp{name}")
                for idx, (xt_sb, w_sb, b0, xpT, j, tc5) in enumerate(jobs):
                    for i in range(KT):
                        nc.tensor.matmul(
                            psP[:, idx, :],
                            w_sb[:, i, j * P:(j + 1) * P],
                            xt_sb[:, i, tc5 * QC:(tc5 + 1) * QC],
                            start=(i == 0),
                            stop=(i == KT - 1),
                        )
                for idx, (xt_sb, w_sb, b0, xpT, j, tc5) in enumerate(jobs):
                    nc.vector.tensor_tensor(
                        xpT[:, j, tc5 * QC:(tc5 + 1) * QC],
                        psP[:, idx, :],
                        bias_sb[:, b0 + j:b0 + j + 1].to_broadcast((P, QC)),
                        ADD,
                    )

            kj = lambda j, t: (kt_sb, wk_sb, 2, kpT, j, t)
            qj = lambda j, t: (qt_sb, wq_sb, 0, qpT, j, t)
            proj_pair([kj(0, 0), kj(0, 1)], "k00")
            proj_pair([kj(0, 2), kj(0, 3)], "k01")
            proj_pair([qj(0, 0), qj(1, 0)], "q0")

            # ---- filler chunk emitters (one [128,512] psF tile each) ----
            def k_chunk(j, tc5):
                def emit():
                    psP = psF.tile([P, QC], F32, tag="f", name=f"kf{j}_{tc5}")
                    for i in range(KT):
                        nc.tensor.matmul(
                            psP[:],
                            wk_sb[:, i, j * P:(j + 1) * P],
                            kt_sb[:, i, tc5 * QC:(tc5 + 1) * QC],
                            start=(i == 0),
                            stop=(i == KT - 1),
                        )
                    nc.vector.tensor_tensor(
                        kpT[:, j, tc5 * QC:(tc5 + 1) * QC],
                        psP[:],
                        bias_sb[:, 2 + j:2 + j + 1].to_broadcast((P, QC)),
                        ADD,
                    )
                return emit

            def q_chunk(j, tc5):
                def emit():
                    psP = psF.tile([P, QC], F32, tag="f", name=f"qf{j}_{tc5}")
                    for i in range(KT):
                        nc.tensor.matmul(
                            psP[:],
                            wq_sb[:, i, j * P:(j + 1) * P],
                            qt_sb[:, i, tc5 * QC:(tc5 + 1) * QC],
                            start=(i == 0),
                            stop=(i == KT - 1),
                        )
                    nc.vector.tensor_tensor(
                        qpT[:, j, tc5 * QC:(tc5 + 1) * QC],
                        psP[:],
                        bias_sb[:, j:j + 1].to_broadcast((P, QC)),
                        ADD,
                    )
                return emit

            def cproj_chunk(mt, cc):
                def emit():
                    o_ps = psF.tile([P, QC], F32, tag="f", name=f"o{mt}_{cc}")
                    for j in range(JT):
                        nc.tensor.matmul(
                            o_ps[:],
                            yallT[:, j, mt * P:(mt + 1) * P],
                            wc_sb[:, j, cc * QC:(cc + 1) * QC],
                            start=(j == 0),
                            stop=(j == JT - 1),
                        )
                    o_sb = opool.tile([P, QC], BF16, tag="osb",
                                      name=f"ot{mt}_{cc}")
                    nc.vector.tensor_copy(o_sb[:], o_ps[:])
                    nc.sync.dma_start(
                        out.ap()[mt * P:(mt + 1) * P, cc * QC:(cc + 1) * QC],
                        o_sb[:],
                    )
                return emit

            def cproj_pair(mt):
                # post-attention: both halves of a row-block via one psS tile
                psP = psS.tile([P, 2, QC], F32, tag="ps", name=f"cp{mt}")
                for cc in range(2):
                    for j in range(JT):
                        nc.tensor.matmul(
                            psP[:, cc, :],
                            yallT[:, j, mt * P:(mt + 1) * P],
                            wc_sb[:, j, cc * QC:(cc + 1) * QC],
                            start=(j == 0),
                            stop=(j == JT - 1),
                        )
                o_sb = opool.tile([P, 2, QC], BF16, tag="osb2", name=f"otp{mt}")
                nc.scalar.copy(o_sb[:], psP[:])
                nc.sync.dma_start(out.ap()[mt * P:(mt + 1) * P, :], o_sb[:])

            # ---- attention pass with fillers ----
            def attn_pass(j, qc, fillers):
                q0 = qc * QC

                def s_mms(mv):
                    pair = []
                    for ab in range(2):
                        s = psS.tile([P, 2, QC], F32, tag="ps",
                                     name=f"s{j}{qc}{mv}{ab}")
                        p0 = ab * HD
                        for sub in range(2):
                            kv = mv * 2 + sub
                            nc.tensor.matmul(
                                s[:, sub, :],
                                kpT[p0:p0 + HD, j, kv * P:(kv + 1) * P],
                                qpT[p0:p0 + HD, j, q0:q0 + QC],
                                start=True,
                                stop=True,
                                tile_position=(p0, 0),
                            )
                        pair.append(s)
                    return pair

                yps = [psY.tile([HD + 1, QC], F32, tag="y",
                                name=f"y{j}{qc}{ab}") for ab in range(2)]
                fl = list(fillers)
                fidx = 0
                pend = s_mms(0)
                for mv in range(NMV):
                    nxt = s_mms(mv + 1) if mv + 1 < NMV else None
                    for ab in range(2):
                        es = epool.tile([P, 2, QC], F32R, tag="es",
                                        name=f"e{j}{qc}{mv}{ab}")
                        nc.scalar.activation(es[:], pend[ab][:], EXP, scale=SCALE)
                        for sub in range(2):
                            kv = mv * 2 + sub
                            nc.tensor.matmul(
                                yps[ab][:],
                                vext[2 * j + ab][:, kv, :],
                                es[:, sub, :],
                                start=(mv == 0 and sub == 0),
                                stop=(mv == NMV - 1 and sub == 1),
                            )
                    pend = nxt
                    if fidx < len(fl):
                        fl[fidx]()
                        fidx += 1
                while fidx < len(fl):
                    fl[fidx]()
                    fidx += 1
                # normalize both heads of the pair
                for ab in range(2):
                    p0 = ab * HD
                    y_ps = yps[ab]
                    dsc = npool.tile([HD + 1, QC], F32, tag="dsc",
                                     name=f"d{j}{qc}{ab}")
                    nc.vector.tensor_copy(dsc[HD:HD + 1, :], y_ps[HD:HD + 1, :])
                    dn0 = bpool.tile([1, QC], F32, tag="dn0",
                                     name=f"dn{j}{qc}{ab}")
                    nc.sync.dma_start(dn0[:], dsc[HD:HD + 1, :])
                    rc = bpool.tile([1, QC], F32, tag="rc",
                                    name=f"rc{j}{qc}{ab}")
                    nc.vector.reciprocal_approx_fast(rc[:], dn0[:])
                    bc = bpool.tile([HD, QC], F32, tag="bc",
                                    name=f"b{j}{qc}{ab}")
                    nc.gpsimd.partition_broadcast(bc[:], rc[:])
                    nc.vector.tensor_tensor(
                        yallT[p0:p0 + HD, j, q0:q0 + QC],
                        y_ps[0:HD, :], bc[:], MULT,
                    )

            # ---- filler schedule over the 8 passes ----
            filler_lists = [
                [k_chunk(1, 0), k_chunk(1, 1), k_chunk(1, 2), k_chunk(1, 3),
                 q_chunk(0, 1)],
                [q_chunk(1, 1), q_chunk(0, 2), q_chunk(1, 2)],
                [cproj_chunk(0, 0), cproj_chunk(0, 1),
                 cproj_chunk(1, 0), cproj_chunk(1, 1)],
                [cproj_chunk(2, 0), cproj_chunk(2, 1),
                 cproj_chunk(3, 0), cproj_chunk(3, 1)],
                [q_chunk(0, 3), q_chunk(1, 3),
                 cproj_chunk(4, 0), cproj_chunk(4, 1)],
                [cproj_chunk(5, 0), cproj_chunk(5, 1),
                 cproj_chunk(6, 0), cproj_chunk(6, 1)],
                [cproj_chunk(7, 0), cproj_chunk(7, 1),
                 cproj_chunk(8, 0), cproj_chunk(8, 1)],
                [cproj_chunk(9, 0), cproj_chunk(9, 1),
                 cproj_chunk(10, 0), cproj_chunk(10, 1)],
            ]

            pi = 0
            for qc in range(NQC):
                for j in range(JT):
                    attn_pass(j, qc, filler_lists[pi])
                    pi += 1

            # tail: remaining cproj row-blocks
            for mt in (11, 12, 13, 14, 15):
                cproj_pair(mt)

    nc.compile()
    _cache["nc"] = nc
    return nc


def _bf(x):
    return np.ascontiguousarray(np.asarray(x, dtype=ml_dtypes.bfloat16))


def make_in_maps(k, q, v, Wk, bk, Wq, bq, Wc, bc):
    k = np.asarray(k, dtype=np.float32)
    q = np.asarray(q, dtype=np.float32)
    v = np.asarray(v, dtype=np.float32)
    Wk = np.asarray(Wk, dtype=np.float32)
    Wq = np.asarray(Wq, dtype=np.float32)
    Wc = np.asarray(Wc, dtype=np.float32)
    bk = np.asarray(bk, dtype=np.float32)
    bq = np.asarray(bq, dtype=np.float32)
    in_maps = []
    for c in range(N_CORES):
        b = c // 4
        h0 = (c % 4) * HL
        sl = slice(h0 * HD, h0 * HD + DH)
        bq_t = np.ascontiguousarray(bq[sl].reshape(2, P).T)  # [128, 2]
        bk_t = np.ascontiguousarray(bk[sl].reshape(2, P).T)
        bqk_m = np.concatenate([bq_t, bk_t], axis=1)         # [128, 4]
        in_maps.append({
            "qT": _bf(q[b].T),
            "kT": _bf(k[b].T),
            "v_sl": np.ascontiguousarray(v[b][:, sl]),
            "WqT": _bf(Wq[sl, :].T),
            "WkT": _bf(Wk[sl, :].T),
            "WcT": _bf(Wc[:, sl].T),
            "bqk": np.ascontiguousarray(bqk_m),
            "onescol": np.ones((P, T // P), dtype=np.float32),
        })
    return in_maps


def kernel(k, q, v, Wk, bk, Wq, bq, Wc, bc, _trace=False, _trace_cores=None):
    bc = np.asarray(bc, dtype=np.float32)
    nc = build_nc()
    in_maps = make_in_maps(k, q, v, Wk, bk, Wq, bq, Wc, bc)
    res = run_bass_kernel_spmd(
        nc, in_maps, core_ids=list(range(N_CORES)),
        trace=_trace, trace_cores=_trace_cores,
    )
    outs = [res.results[c]["out"].astype(np.float32) for c in range(N_CORES)]
    full = np.stack([
        outs[0] + outs[1] + outs[2] + outs[3],
        outs[4] + outs[5] + outs[6] + outs[7],
    ]) + bc[None, None, :]
    kernel.last_result = res
    return full.astype(np.float32)
